# revision 15
# baseline (speedup 1.0000x reference)
"""Trainium2 Bass kernel for AFT-style sparse attention (nn_AFTKVR).

Reference computation (per batch b):
    q,k,v = x @ W{q,k,v}.T          # [T=1024, D=256], H=4 heads x d=64
    ew = exp(wbias)                  # [T, K=63] neighbor weights
    ek = exp(k); kv = ek * v
    num[t] = sum_k ew[t,k] * kv[idx[t,k]]   (idx = row+col neighbors on 32x32 grid)
    den[t] = sum_k ew[t,k] * ek[idx[t,k]]
    out = sigmoid(q) * num / den

Sharding: 8 cores = 4 batches x 2 head-pairs (128 features each). No collectives.

Per-core device algorithm (all matmul operands bf16, fp32 PSUM accumulation):
  - inputs stream on one queue in need-order: [wq|wkv] (192KB), xt in 4
    quarter transfers (kh x token-half), then the host-expanded
    block-diagonal neighbor weights wrow_e / wcol_e (256KB each, which
    hide behind the PE phase).  xt cols = kh*1024 + t.
  - the grid-transposed xt2 (token t' = c*32+r) is built ON-DEVICE by 4
    strided DVE copies, each gated on one xt quarter -- no HBM traffic.
  - q projected feature-major -> qT PSUM; ACT computes eq = exp(-qT).
  - k|v projected token-major per 128-token group (lhsT = xt slice) ->
    ek, kv; same from xt2 -> ekp, kvp (grid-col-major blocks).
  - The 63-neighbor gather+reduce decomposes into block-diagonal matmuls
    accumulated into zeroed PSUM (start=False + skip_group_check):
      row part: numT[f, tok-slice] += kv_g.T @ wrow_g   (16 matmuls)
      col part: numT[f, strided]   += kvp_g.T @ wcol_g  (16 matmuls,
                den first so the combine's recip chain overlaps them)
  - combine (DVE): m1 = (eq+1)*den [scalar_tensor_tensor] and
    rden = recip_approx(m1) for all quarters run DURING the col-num
    matmuls; only the final muls out = num*rden trail the PE
    (== sigmoid(q)*num/den).  Written feature-major bf16; host
    casts/transposes during unshard.  Output DMAs alternate between the
    sync and scalar DGE queues so descriptor issue is not serialized.
  - dummy matmuls warm the PE HAM clock gate while inputs stream in; a
    dummy Exp preloads the ACT table so no table load sits mid-kernel.
"""

import os
from contextlib import ExitStack

import ml_dtypes
import numpy as np

import concourse.bass as bass
import concourse.tile as tile
from concourse import bacc, mybir
from concourse.bass_utils import run_bass_kernel_spmd

BF = mybir.dt.bfloat16
F32 = mybir.dt.float32
AF = mybir.ActivationFunctionType
ALU = mybir.AluOpType

N = 32          # grid side
T = N * N       # tokens
D = 256         # model dim
F = 128         # features per core (2 heads x 64)
NEG = -1e30     # exp(NEG) == 0

LAST_RESULT = None  # BassKernelResults of the most recent run (for profiling)
_CACHED_NC = None


def _build_nc():
    nc = bacc.Bacc("TRN2", target_bir_lowering=False, debug=False)

    xt_ds = [nc.declare_dram_parameter(f"xt{i}", [128, 512], BF, isOutput=False)
             for i in range(4)]
    wb1_d = nc.declare_dram_parameter("wb1", [128, 768], BF, isOutput=False)
    wrow_d = nc.declare_dram_parameter("wrow", [128, 1024], BF, isOutput=False)
    wcol_d = nc.declare_dram_parameter("wcol", [128, 1024], BF, isOutput=False)
    out_d = nc.declare_dram_parameter("out", [128, 1024], BF, isOutput=True)

    from concourse.tile_rust import add_dep_helper

    with tile.TileContext(nc) as tc, ExitStack() as ctx:
        sb = ctx.enter_context(tc.tile_pool(name="sb", bufs=1))
        ps_q = ctx.enter_context(tc.tile_pool(name="ps_q", bufs=1, space="PSUM"))
        ps_kv = ctx.enter_context(tc.tile_pool(name="ps_kv", bufs=3, space="PSUM"))
        ps_g = ctx.enter_context(tc.tile_pool(name="ps_g", bufs=1, space="PSUM"))

        xt = sb.tile([128, 2048], BF, tag="xt")
        xt2 = sb.tile([128, 2048], BF, tag="xt2")
        wb1 = sb.tile([128, 768], BF, tag="wb1")
        wrow_e = sb.tile([128, 1024], BF, tag="wrow_e")
        wcol_e = sb.tile([128, 1024], BF, tag="wcol_e")
        warm = sb.tile([128, 512], BF, tag="warm")
        pre = sb.tile([128, 16], F32, tag="pre")
        ek = sb.tile([128, 1024], BF, tag="ek")
        kv = sb.tile([128, 1024], BF, tag="kv")
        ekp = sb.tile([128, 1024], BF, tag="ekp")
        kvp = sb.tile([128, 1024], BF, tag="kvp")
        eq = sb.tile([128, 1024], F32, tag="eq")
        m1 = sb.tile([128, 1024], F32, tag="m1")
        rden = sb.tile([128, 1024], F32, tag="rden")
        w2 = sb.tile([128, 1024], BF, tag="w2")

        wq = wb1[:, 0:256]
        wkv = wb1[:, 256:768]

        # input loads split across BOTH hardware DGE queues (sync -> Q1,
        # scalar -> Q10) so the two streams run in parallel, each in
        # consumption-priority order.  xt quarter i covers cols i*512..
        # (i<2: kh=0 token halves, i>=2: kh=1).
        nc.scalar.dma_start(out=xt[:, 1024:1536], in_=xt_ds[2][:])
        nc.scalar.dma_start(out=xt[:, 1536:2048], in_=xt_ds[3][:])
        nc.scalar.dma_start(out=wcol_e[:], in_=wcol_d[:])
        nc.sync.dma_start(out=wb1[:], in_=wb1_d[:])
        nc.sync.dma_start(out=xt[:, 0:512], in_=xt_ds[0][:])
        nc.sync.dma_start(out=xt[:, 512:1024], in_=xt_ds[1][:])
        nc.sync.dma_start(out=wrow_e[:], in_=wrow_d[:])

        # PE warm-up: dummy matmuls while the input DMAs stream in, so the
        # HAM clock gate is released (1.2 -> 2.4 GHz) before the real work
        nc.gpsimd.memset(warm[:], 0.0)
        for i in range(7):
            # 4 big then 3 small dummy matmuls: keep the PE continuously
            # busy from engine start until the first xt quarter lands, so
            # the clock-ramp activity window never resets
            wps = ps_kv.tile([128, 512], F32, tag="kvps")
            w_ = 512 if i < 4 else 128
            nc.tensor.matmul(wps[:, 0:w_], warm[:, 0:128], warm[:, 0:w_],
                             start=True, stop=True)

        # ACT table preload: a dummy Exp so the (only) table load happens
        # while inputs stream in
        nc.scalar.activation(pre[:], warm[:, 0:16], AF.Exp)

        # zero the grid accumulators on the (idle-early) DVE; every grid
        # matmul then accumulates with start=False + skip_group_check.
        # (den first -- its memsets gate the earliest grid matmuls; the
        # num memsets are interleaved after the xt2 copies below)
        numT = ps_g.tile([128, 1024], F32, tag="numT")
        denT = ps_g.tile([128, 1024], F32, tag="denT")
        for bank in range(2):
            nc.vector.memset(denT[:, bank * 512:(bank + 1) * 512], 0.0)

        # on-device grid transpose xt -> xt2 (t' = c*32 + r): 4 strided DVE
        # copies, each gated on one xt quarter transfer
        xt_cr = xt[:].rearrange("p (kh r c) -> p kh c r", kh=2, r=N)
        xt2_cr = xt2[:].rearrange("p (kh c r) -> p kh c r", kh=2, c=N)

        ek_vw = ek[:].rearrange("p (g f) -> p g f", f=128)
        kv_vw = kv[:].rearrange("p (g f) -> p g f", f=128)
        ekp_vw = ekp[:].rearrange("p (g f) -> p g f", f=128)
        kvp_vw = kvp[:].rearrange("p (g f) -> p g f", f=128)

        qp = {}

        def q_proj(nh):
            qp[nh] = ps_q.tile([128, 512], F32, name=f"qp{nh}", tag="qp")
            for kh in range(2):
                nc.tensor.matmul(
                    qp[nh][:],
                    wq[:, kh * 128:(kh + 1) * 128],
                    xt[:, kh * 1024 + nh * 512: kh * 1024 + (nh + 1) * 512],
                    start=(kh == 0), stop=(kh == 1),
                )

        def kv_proj(pr, src, ek_t, kv_t, do_mul=True):
            kvps = ps_kv.tile([128, 512], F32, tag="kvps")
            mm = {}
            for g2 in range(2):
                g = 2 * pr + g2
                for kh in range(2):
                    lhsT = src[:, kh * 1024 + g * 128: kh * 1024 + (g + 1) * 128]
                    mm[g2, kh] = nc.tensor.matmul(
                        kvps[:, g2 * 256:(g2 + 1) * 256],
                        lhsT,
                        wkv[:, kh * 256:(kh + 1) * 256],
                        start=(g2 == 0 and kh == 0),
                        stop=(g2 == 1 and kh == 1),
                    )
            # keep PSUM zero-region state machine ordering legal: the
            # start=True matmul first, the stop=True matmul last
            add_dep_helper(mm[1, 0].ins, mm[0, 0].ins, reason="psum start first")
            add_dep_helper(mm[1, 1].ins, mm[0, 1].ins, reason="psum stop last")
            kvps_v = kvps[:].rearrange("p (g c) -> p g c", g=2)
            ps_ = slice(2 * pr, 2 * pr + 2)
            nc.scalar.activation(ek_t[:, ps_, :], kvps_v[:, :, 0:128], AF.Exp)
            nc.vector.tensor_mul(kv_t[:, ps_, :], ek_t[:, ps_, :],
                                 kvps_v[:, :, 128:256])

        # PE order: q0 -> kvA pr0-1 -> kvA pr2-3 -> q1 -> kvB -> row -> col.
        # ACT order: expA0, eq0, expA1-3, eq1, expB0-3.
        # DVE order: memsets, xt2 copies + muls A interleaved, muls B,
        #            stt+recip (all quarters), final muls.
        q_proj(0)
        kv_proj(0, xt, ek_vw, kv_vw)
        nc.scalar.activation(eq[:, 0:512], qp[0][:], AF.Exp, scale=-1.0)
        # xt2 copies, in expected quarter-arrival order
        nc.vector.tensor_copy(xt2_cr[:, 1, :, 0:16], xt_cr[:, 1, :, 0:16])
        nc.vector.tensor_copy(xt2_cr[:, 1, :, 16:32], xt_cr[:, 1, :, 16:32])
        kv_proj(1, xt, ek_vw, kv_vw)
        nc.vector.tensor_copy(xt2_cr[:, 0, :, 0:16], xt_cr[:, 0, :, 0:16])
        kv_proj(2, xt, ek_vw, kv_vw)
        nc.vector.tensor_copy(xt2_cr[:, 0, :, 16:32], xt_cr[:, 0, :, 16:32])
        for bank in range(2):
            nc.vector.memset(numT[:, bank * 512:(bank + 1) * 512], 0.0)
        kv_proj(3, xt, ek_vw, kv_vw)
        q_proj(1)
        for pr in range(4):
            kv_proj(pr, xt2, ekp_vw, kvp_vw)
        nc.scalar.activation(eq[:, 512:1024], qp[1][:], AF.Exp, scale=-1.0)

        # grid reduction, den parts FIRST (row den, col den, row num,
        # col num) so the combine's den->recip chain overlaps the num
        # matmuls.  Row part writes contiguous out cols per 4-grid-row
        # slice; col part writes strided out cols (token r*32+c).
        GK = dict(start=False, stop=False, skip_group_check=True)
        numT_v = numT[:].rearrange("p (r c) -> p c r", c=N)
        denT_v = denT[:].rearrange("p (r c) -> p c r", c=N)
        wcol_gv = wcol_e[:].rearrange("p (g cb r) -> p g cb r", g=8, cb=4)
        for g in range(8):
            gs = slice(g * 128, (g + 1) * 128)
            nc.tensor.matmul(denT[:, gs], ek[:, gs], wrow_e[:, gs], **GK)
        for g in range(8):
            gs = slice(g * 128, (g + 1) * 128)
            nc.tensor.matmul(denT_v[:, 4 * g:4 * (g + 1), :],
                             ekp[:, gs], wcol_gv[:, g], **GK)
        for g in range(8):
            gs = slice(g * 128, (g + 1) * 128)
            nc.tensor.matmul(numT[:, gs], kv[:, gs], wrow_e[:, gs], **GK)
        for g in range(8):
            gs = slice(g * 128, (g + 1) * 128)
            nc.tensor.matmul(numT_v[:, 4 * g:4 * (g + 1), :],
                             kvp[:, gs], wcol_gv[:, g], **GK)

        # combine: out = num * recip(den * (1 + exp(-q))) == sigmoid(q)*num/den
        # in halves: stt+recip (den-gated) overlap the num matmuls; only
        # the two final muls trail the PE, each feeding its output DMA.
        for hf in range(2):
            hs = slice(hf * 512, (hf + 1) * 512)
            nc.vector.scalar_tensor_tensor(
                m1[:, hs], eq[:, hs], 1.0, denT[:, hs], ALU.add, ALU.mult)
            nc.vector.reciprocal_approx_fast(rden[:, hs], m1[:, hs])
        for qt in range(4):
            hs = slice(qt * 256, (qt + 1) * 256)
            nc.vector.tensor_mul(w2[:, hs], rden[:, hs], numT[:, hs])
            eng = nc.sync if qt % 2 == 0 else nc.scalar
            eng.dma_start(out=out_d[:, hs], in_=w2[:, hs])

    nc.compile()
    return nc


def _get_nc():
    global _CACHED_NC
    if _CACHED_NC is None:
        _CACHED_NC = _build_nc()
    return _CACHED_NC


def _interleave_halves(a):
    """[256, M] -> [128, 2*M] with cols (half, m); partitions = dim%128."""
    return np.concatenate([a[0:128], a[128:256]], axis=1)


def make_shards(x, Wq, Wk, Wv, wbias):
    """Build the per-core input maps (host-side layout/sharding only)."""
    bf = ml_dtypes.bfloat16
    B = x.shape[0]

    # neighbor-weight reorganization: for token t=(r,c), sorted wbias cols are
    #   [0, r)   -> col-neighbor grid-row j = pos
    #   [r, r+N) -> row-neighbor grid-col j = pos - r
    #   [r+N, 2N-1) -> col-neighbor grid-row j = pos - (N - 1)
    Wr = np.empty((T, N), np.float32)
    Wc = np.full((T, N), NEG, np.float32)
    for t in range(T):
        r = t // N
        Wr[t] = wbias[t, r:r + N]
        Wc[t, :r] = wbias[t, :r]
        Wc[t, r + 1:] = wbias[t, r + N:]

    # block-diagonal expanded weights (exp-folded; off-block entries 0):
    #   wrow[rb*32+j, g*128 + rb*32 + c] = exp(Wr[(4g+rb)*32 + c, j])
    #   wcol[cb*32+j, g*128 + cb*32 + r] = exp(Wc[r*32 + 4g+cb, j])
    wrow = np.full((128, 1024), NEG, np.float32)
    wcol = np.full((128, 1024), NEG, np.float32)
    rb, j, c = np.meshgrid(np.arange(4), np.arange(N), np.arange(N),
                           indexing="ij")
    for g in range(8):
        wrow[rb * N + j, g * 128 + rb * N + c] = Wr[(4 * g + rb) * N + c, j]
        wcol[rb * N + j, g * 128 + rb * N + c] = Wc[c * N + 4 * g + rb, j]
    wrow = np.exp(wrow).astype(bf)
    wcol = np.exp(wcol).astype(bf)

    xt_b = []
    for b in range(B):
        xtb = np.ascontiguousarray(_interleave_halves(x[b].T)).astype(bf)
        xt_b.append(xtb)

    in_maps = []
    for core in range(8):
        b, hp = core // 2, core % 2
        sl = slice(hp * 128, (hp + 1) * 128)
        wq_c = _interleave_halves(Wq[sl].T).astype(bf)                # [128,256]
        k_h = Wk[sl].T.reshape(2, 128, 128)
        v_h = Wv[sl].T.reshape(2, 128, 128)
        wkv_c = np.concatenate([k_h[0], v_h[0], k_h[1], v_h[1]],
                               axis=1).astype(bf)                      # [128,512]
        wb1_c = np.ascontiguousarray(
            np.concatenate([wq_c, wkv_c], axis=1))                     # [128,768]
        in_maps.append({
            "xt0": np.ascontiguousarray(xt_b[b][:, 0:512]),
            "xt1": np.ascontiguousarray(xt_b[b][:, 512:1024]),
            "xt2": np.ascontiguousarray(xt_b[b][:, 1024:1536]),
            "xt3": np.ascontiguousarray(xt_b[b][:, 1536:2048]),
            "wb1": wb1_c,
            "wrow": wrow,
            "wcol": wcol,
        })
    return in_maps


def kernel(x, Wq, Wk, Wv, wbias, key_indices=None, **_unused):
    global LAST_RESULT
    x = np.asarray(x, np.float32)
    Wq = np.asarray(Wq, np.float32)
    Wk = np.asarray(Wk, np.float32)
    Wv = np.asarray(Wv, np.float32)
    wbias = np.asarray(wbias, np.float32)

    nc = _get_nc()
    in_maps = make_shards(x, Wq, Wk, Wv, wbias)
    try:
        res = run_bass_kernel_spmd(nc, in_maps, core_ids=list(range(8)))
    except ModuleNotFoundError:
        # BASS_TRACE set but the NTFF profile hook module is unavailable in
        # this environment -- rerun untraced
        os.environ["BASS_NEVER_TRACE"] = "1"
        res = run_bass_kernel_spmd(nc, in_maps, core_ids=list(range(8)))
    LAST_RESULT = res

    B = x.shape[0]
    out = np.empty((B, T, D), np.float32)
    for core in range(8):
        b, hp = core // 2, core % 2
        out[b, :, hp * 128:(hp + 1) * 128] = \
            res.results[core]["out"].astype(np.float32).T
    return out


# revision 19
# speedup vs baseline: 1.0055x; 1.0055x over previous
"""Trainium2 Bass kernel for AFT-style sparse attention (nn_AFTKVR).

Reference computation (per batch b):
    q,k,v = x @ W{q,k,v}.T          # [T=1024, D=256], H=4 heads x d=64
    ew = exp(wbias)                  # [T, K=63] neighbor weights
    ek = exp(k); kv = ek * v
    num[t] = sum_k ew[t,k] * kv[idx[t,k]]   (idx = row+col neighbors on 32x32 grid)
    den[t] = sum_k ew[t,k] * ek[idx[t,k]]
    out = sigmoid(q) * num / den

Sharding: 8 cores = 4 batches x 2 head-pairs (128 features each). No collectives.

Per-core device algorithm (all matmul operands bf16, fp32 PSUM accumulation):
  - inputs stream on one queue in need-order: [wq|wkv] (192KB), xt in 4
    quarter transfers (kh x token-half), then the host-expanded
    block-diagonal neighbor weights wrow_e / wcol_e (256KB each, which
    hide behind the PE phase).  xt cols = kh*1024 + t.
  - the grid-transposed xt2 (token t' = c*32+r) is built ON-DEVICE by 4
    strided DVE copies, each gated on one xt quarter -- no HBM traffic.
  - q projected feature-major -> qT PSUM; ACT computes eq = exp(-qT).
  - k|v projected token-major per 128-token group (lhsT = xt slice) ->
    ek, kv; same from xt2 -> ekp, kvp (grid-col-major blocks).
  - The 63-neighbor gather+reduce decomposes into block-diagonal matmuls
    accumulated into zeroed PSUM (start=False + skip_group_check):
      row part: numT[f, tok-slice] += kv_g.T @ wrow_g   (16 matmuls)
      col part: numT[f, strided]   += kvp_g.T @ wcol_g  (16 matmuls,
                den first so the combine's recip chain overlaps them)
  - combine (DVE): m1 = (eq+1)*den [scalar_tensor_tensor] and
    rden = recip_approx(m1) for all quarters run DURING the col-num
    matmuls; only the final muls out = num*rden trail the PE
    (== sigmoid(q)*num/den).  Written feature-major bf16; host
    casts/transposes during unshard.  Output DMAs alternate between the
    sync and scalar DGE queues so descriptor issue is not serialized.
  - dummy matmuls warm the PE HAM clock gate while inputs stream in; a
    dummy Exp preloads the ACT table so no table load sits mid-kernel.
"""

import os
from contextlib import ExitStack

import ml_dtypes
import numpy as np

import concourse.bass as bass
import concourse.tile as tile
from concourse import bacc, mybir
from concourse.bass_utils import run_bass_kernel_spmd

BF = mybir.dt.bfloat16
F32 = mybir.dt.float32
AF = mybir.ActivationFunctionType
ALU = mybir.AluOpType

N = 32          # grid side
T = N * N       # tokens
D = 256         # model dim
F = 128         # features per core (2 heads x 64)
NEG = -1e30     # exp(NEG) == 0

LAST_RESULT = None  # BassKernelResults of the most recent run (for profiling)
_CACHED_NC = None


def _build_nc():
    nc = bacc.Bacc("TRN2", target_bir_lowering=False, debug=False)

    xt_ds = [nc.declare_dram_parameter(f"xt{i}", [128, 512], BF, isOutput=False)
             for i in range(4)]
    wb1_d = nc.declare_dram_parameter("wb1", [128, 768], BF, isOutput=False)
    wrow_d = nc.declare_dram_parameter("wrow", [128, 1024], BF, isOutput=False)
    wcol_d = nc.declare_dram_parameter("wcol", [128, 1024], BF, isOutput=False)
    out_d = nc.declare_dram_parameter("out", [128, 1024], BF, isOutput=True)

    from concourse.tile_rust import add_dep_helper

    with tile.TileContext(nc) as tc, ExitStack() as ctx:
        sb = ctx.enter_context(tc.tile_pool(name="sb", bufs=1))
        ps_q = ctx.enter_context(tc.tile_pool(name="ps_q", bufs=1, space="PSUM"))
        ps_kv = ctx.enter_context(tc.tile_pool(name="ps_kv", bufs=3, space="PSUM"))
        ps_g = ctx.enter_context(tc.tile_pool(name="ps_g", bufs=1, space="PSUM"))

        xt = sb.tile([128, 2048], BF, tag="xt")
        xt2 = sb.tile([128, 2048], BF, tag="xt2")
        wb1 = sb.tile([128, 768], BF, tag="wb1")
        wrow_e = sb.tile([128, 1024], BF, tag="wrow_e")
        wcol_e = sb.tile([128, 1024], BF, tag="wcol_e")
        warm = sb.tile([128, 512], BF, tag="warm")
        pre = sb.tile([128, 16], F32, tag="pre")
        ek = sb.tile([128, 1024], BF, tag="ek")
        kv = sb.tile([128, 1024], BF, tag="kv")
        ekp = sb.tile([128, 1024], BF, tag="ekp")
        kvp = sb.tile([128, 1024], BF, tag="kvp")
        eq = sb.tile([128, 1024], F32, tag="eq")
        m1 = sb.tile([128, 1024], F32, tag="m1")
        rden = sb.tile([128, 1024], F32, tag="rden")
        w2 = sb.tile([128, 1024], BF, tag="w2")

        wq = wb1[:, 0:256]
        wkv = wb1[:, 256:768]

        # input loads split across BOTH hardware DGE queues (sync -> Q1,
        # scalar -> Q10) so the two streams run in parallel, each in
        # consumption-priority order.  xt quarter i covers cols i*512..
        # (i<2: kh=0 token halves, i>=2: kh=1).
        nc.scalar.dma_start(out=xt[:, 1024:1536], in_=xt_ds[2][:])
        nc.scalar.dma_start(out=xt[:, 1536:2048], in_=xt_ds[3][:])
        nc.scalar.dma_start(out=wcol_e[:], in_=wcol_d[:])
        nc.sync.dma_start(out=wb1[:], in_=wb1_d[:])
        nc.sync.dma_start(out=xt[:, 0:512], in_=xt_ds[0][:])
        nc.sync.dma_start(out=xt[:, 512:1024], in_=xt_ds[1][:])
        nc.sync.dma_start(out=wrow_e[:], in_=wrow_d[:])

        # grid-transpose copies for the kh=1 half of xt2 run on the ACT
        # engine's idle early window (their sources arrive first, on Q10);
        # the kh=0 half runs on DVE.  This keeps the DVE queue light so
        # the ekp/kvp muls (which gate the col-den matmuls and thus the
        # combine chain) run as early as possible.

        # PE warm-up: dummy matmuls while the input DMAs stream in, so the
        # HAM clock gate is released (1.2 -> 2.4 GHz) before the real work
        nc.gpsimd.memset(warm[:], 0.0)
        for i in range(9):
            # 7 medium then 2 small dummy matmuls: keep the PE continuously
            # busy from engine start until the first xt quarter lands, so
            # the clock-ramp activity window never resets
            wps = ps_kv.tile([128, 512], F32, tag="kvps")
            w_ = 256 if i < 7 else 128
            nc.tensor.matmul(wps[:, 0:w_], warm[:, 0:128], warm[:, 0:w_],
                             start=True, stop=True)

        # ACT table preload: a dummy Exp so the (only) table load happens
        # while inputs stream in
        nc.scalar.activation(pre[:], warm[:, 0:16], AF.Exp)

        # zero the grid accumulators on the (idle-early) DVE; every grid
        # matmul then accumulates with start=False + skip_group_check.
        # (den first -- its memsets gate the earliest grid matmuls; the
        # num memsets are interleaved after the xt2 copies below)
        numT = ps_g.tile([128, 1024], F32, tag="numT")
        denT = ps_g.tile([128, 1024], F32, tag="denT")
        for bank in range(2):
            nc.vector.memset(denT[:, bank * 512:(bank + 1) * 512], 0.0)

        # on-device grid transpose xt -> xt2 (t' = c*32 + r): 4 strided DVE
        # copies, each gated on one xt quarter transfer
        xt_cr = xt[:].rearrange("p (kh r c) -> p kh c r", kh=2, r=N)
        xt2_cr = xt2[:].rearrange("p (kh c r) -> p kh c r", kh=2, c=N)

        ek_vw = ek[:].rearrange("p (g f) -> p g f", f=128)
        kv_vw = kv[:].rearrange("p (g f) -> p g f", f=128)
        ekp_vw = ekp[:].rearrange("p (g f) -> p g f", f=128)
        kvp_vw = kvp[:].rearrange("p (g f) -> p g f", f=128)

        qp = {}

        def q_proj(nh):
            qp[nh] = ps_q.tile([128, 512], F32, name=f"qp{nh}", tag="qp")
            for kh in range(2):
                nc.tensor.matmul(
                    qp[nh][:],
                    wq[:, kh * 128:(kh + 1) * 128],
                    xt[:, kh * 1024 + nh * 512: kh * 1024 + (nh + 1) * 512],
                    start=(kh == 0), stop=(kh == 1),
                )

        def kv_proj(pr, src, ek_t, kv_t, do_mul=True):
            kvps = ps_kv.tile([128, 512], F32, tag="kvps")
            mm = {}
            for g2 in range(2):
                g = 2 * pr + g2
                for kh in range(2):
                    lhsT = src[:, kh * 1024 + g * 128: kh * 1024 + (g + 1) * 128]
                    mm[g2, kh] = nc.tensor.matmul(
                        kvps[:, g2 * 256:(g2 + 1) * 256],
                        lhsT,
                        wkv[:, kh * 256:(kh + 1) * 256],
                        start=(g2 == 0 and kh == 0),
                        stop=(g2 == 1 and kh == 1),
                    )
            # keep PSUM zero-region state machine ordering legal: the
            # start=True matmul first, the stop=True matmul last
            add_dep_helper(mm[1, 0].ins, mm[0, 0].ins, reason="psum start first")
            add_dep_helper(mm[1, 1].ins, mm[0, 1].ins, reason="psum stop last")
            kvps_v = kvps[:].rearrange("p (g c) -> p g c", g=2)
            ps_ = slice(2 * pr, 2 * pr + 2)
            nc.scalar.activation(ek_t[:, ps_, :], kvps_v[:, :, 0:128], AF.Exp)
            nc.vector.tensor_mul(kv_t[:, ps_, :], ek_t[:, ps_, :],
                                 kvps_v[:, :, 128:256])

        # PE order: q0 -> kvA pr0-1 -> kvA pr2-3 -> q1 -> kvB -> row -> col.
        # ACT order: expA0, eq0, expA1-3, eq1, expB0-3.
        # DVE order: memsets, xt2 copies + muls A interleaved, muls B,
        #            stt+recip (all quarters), final muls.
        # xt2 copies for kh=1 go FIRST on the ACT queue (their sources are
        # Q10's first two transfers, landing before any exp input exists);
        # the kh=0 copies run on DVE
        nc.scalar.copy(xt2_cr[:, 1, :, 0:16], xt_cr[:, 1, :, 0:16])
        nc.scalar.copy(xt2_cr[:, 1, :, 16:32], xt_cr[:, 1, :, 16:32])
        q_proj(0)
        kv_proj(0, xt, ek_vw, kv_vw)
        nc.scalar.activation(eq[:, 0:512], qp[0][:], AF.Exp, scale=-1.0)
        kv_proj(1, xt, ek_vw, kv_vw)
        nc.vector.tensor_copy(xt2_cr[:, 0, :, 0:16], xt_cr[:, 0, :, 0:16])
        kv_proj(2, xt, ek_vw, kv_vw)
        nc.vector.tensor_copy(xt2_cr[:, 0, :, 16:32], xt_cr[:, 0, :, 16:32])
        for bank in range(2):
            nc.vector.memset(numT[:, bank * 512:(bank + 1) * 512], 0.0)
        kv_proj(3, xt, ek_vw, kv_vw)
        q_proj(1)
        for pr in range(4):
            kv_proj(pr, xt2, ekp_vw, kvp_vw)
        nc.scalar.activation(eq[:, 512:1024], qp[1][:], AF.Exp, scale=-1.0)

        # grid reduction, den parts FIRST (row den, col den, row num,
        # col num) so the combine's den->recip chain overlaps the num
        # matmuls.  Row part writes contiguous out cols per 4-grid-row
        # slice; col part writes strided out cols (token r*32+c).
        GK = dict(start=False, stop=False, skip_group_check=True)
        numT_v = numT[:].rearrange("p (r c) -> p c r", c=N)
        denT_v = denT[:].rearrange("p (r c) -> p c r", c=N)
        wcol_gv = wcol_e[:].rearrange("p (g cb r) -> p g cb r", g=8, cb=4)
        for g in range(8):
            gs = slice(g * 128, (g + 1) * 128)
            nc.tensor.matmul(denT[:, gs], ek[:, gs], wrow_e[:, gs], **GK)
        for g in range(8):
            gs = slice(g * 128, (g + 1) * 128)
            nc.tensor.matmul(denT_v[:, 4 * g:4 * (g + 1), :],
                             ekp[:, gs], wcol_gv[:, g], **GK)
        for g in range(8):
            gs = slice(g * 128, (g + 1) * 128)
            nc.tensor.matmul(numT[:, gs], kv[:, gs], wrow_e[:, gs], **GK)
        for g in range(8):
            gs = slice(g * 128, (g + 1) * 128)
            nc.tensor.matmul(numT_v[:, 4 * g:4 * (g + 1), :],
                             kvp[:, gs], wcol_gv[:, g], **GK)

        # combine: out = num * recip(den * (1 + exp(-q))) == sigmoid(q)*num/den
        # in halves: stt+recip (den-gated) overlap the num matmuls; only
        # the two final muls trail the PE, each feeding its output DMA.
        for hf in range(2):
            hs = slice(hf * 512, (hf + 1) * 512)
            nc.vector.scalar_tensor_tensor(
                m1[:, hs], eq[:, hs], 1.0, denT[:, hs], ALU.add, ALU.mult)
            nc.vector.reciprocal_approx_fast(rden[:, hs], m1[:, hs])
        for qt in range(4):
            hs = slice(qt * 256, (qt + 1) * 256)
            nc.vector.tensor_mul(w2[:, hs], rden[:, hs], numT[:, hs])
            eng = nc.sync if qt % 2 == 0 else nc.scalar
            eng.dma_start(out=out_d[:, hs], in_=w2[:, hs])

    nc.compile()
    return nc


def _get_nc():
    global _CACHED_NC
    if _CACHED_NC is None:
        _CACHED_NC = _build_nc()
    return _CACHED_NC


def _interleave_halves(a):
    """[256, M] -> [128, 2*M] with cols (half, m); partitions = dim%128."""
    return np.concatenate([a[0:128], a[128:256]], axis=1)


def make_shards(x, Wq, Wk, Wv, wbias):
    """Build the per-core input maps (host-side layout/sharding only)."""
    bf = ml_dtypes.bfloat16
    B = x.shape[0]

    # neighbor-weight reorganization: for token t=(r,c), sorted wbias cols are
    #   [0, r)   -> col-neighbor grid-row j = pos
    #   [r, r+N) -> row-neighbor grid-col j = pos - r
    #   [r+N, 2N-1) -> col-neighbor grid-row j = pos - (N - 1)
    Wr = np.empty((T, N), np.float32)
    Wc = np.full((T, N), NEG, np.float32)
    for t in range(T):
        r = t // N
        Wr[t] = wbias[t, r:r + N]
        Wc[t, :r] = wbias[t, :r]
        Wc[t, r + 1:] = wbias[t, r + N:]

    # block-diagonal expanded weights (exp-folded; off-block entries 0):
    #   wrow[rb*32+j, g*128 + rb*32 + c] = exp(Wr[(4g+rb)*32 + c, j])
    #   wcol[cb*32+j, g*128 + cb*32 + r] = exp(Wc[r*32 + 4g+cb, j])
    wrow = np.full((128, 1024), NEG, np.float32)
    wcol = np.full((128, 1024), NEG, np.float32)
    rb, j, c = np.meshgrid(np.arange(4), np.arange(N), np.arange(N),
                           indexing="ij")
    for g in range(8):
        wrow[rb * N + j, g * 128 + rb * N + c] = Wr[(4 * g + rb) * N + c, j]
        wcol[rb * N + j, g * 128 + rb * N + c] = Wc[c * N + 4 * g + rb, j]
    wrow = np.exp(wrow).astype(bf)
    wcol = np.exp(wcol).astype(bf)

    xt_b = []
    for b in range(B):
        xtb = np.ascontiguousarray(_interleave_halves(x[b].T)).astype(bf)
        xt_b.append(xtb)

    in_maps = []
    for core in range(8):
        b, hp = core // 2, core % 2
        sl = slice(hp * 128, (hp + 1) * 128)
        wq_c = _interleave_halves(Wq[sl].T).astype(bf)                # [128,256]
        k_h = Wk[sl].T.reshape(2, 128, 128)
        v_h = Wv[sl].T.reshape(2, 128, 128)
        wkv_c = np.concatenate([k_h[0], v_h[0], k_h[1], v_h[1]],
                               axis=1).astype(bf)                      # [128,512]
        wb1_c = np.ascontiguousarray(
            np.concatenate([wq_c, wkv_c], axis=1))                     # [128,768]
        in_maps.append({
            "xt0": np.ascontiguousarray(xt_b[b][:, 0:512]),
            "xt1": np.ascontiguousarray(xt_b[b][:, 512:1024]),
            "xt2": np.ascontiguousarray(xt_b[b][:, 1024:1536]),
            "xt3": np.ascontiguousarray(xt_b[b][:, 1536:2048]),
            "wb1": wb1_c,
            "wrow": wrow,
            "wcol": wcol,
        })
    return in_maps


def kernel(x, Wq, Wk, Wv, wbias, key_indices=None, **_unused):
    global LAST_RESULT
    x = np.asarray(x, np.float32)
    Wq = np.asarray(Wq, np.float32)
    Wk = np.asarray(Wk, np.float32)
    Wv = np.asarray(Wv, np.float32)
    wbias = np.asarray(wbias, np.float32)

    nc = _get_nc()
    in_maps = make_shards(x, Wq, Wk, Wv, wbias)
    try:
        res = run_bass_kernel_spmd(nc, in_maps, core_ids=list(range(8)))
    except ModuleNotFoundError:
        # BASS_TRACE set but the NTFF profile hook module is unavailable in
        # this environment -- rerun untraced
        os.environ["BASS_NEVER_TRACE"] = "1"
        res = run_bass_kernel_spmd(nc, in_maps, core_ids=list(range(8)))
    LAST_RESULT = res

    B = x.shape[0]
    out = np.empty((B, T, D), np.float32)
    for core in range(8):
        b, hp = core // 2, core % 2
        out[b, :, hp * 128:(hp + 1) * 128] = \
            res.results[core]["out"].astype(np.float32).T
    return out


# revision 22
# speedup vs baseline: 1.0377x; 1.0320x over previous
"""Trainium2 Bass kernel for AFT-style sparse attention (nn_AFTKVR).

Reference computation (per batch b):
    q,k,v = x @ W{q,k,v}.T          # [T=1024, D=256], H=4 heads x d=64
    ew = exp(wbias)                  # [T, K=63] neighbor weights
    ek = exp(k); kv = ek * v
    num[t] = sum_k ew[t,k] * kv[idx[t,k]]   (idx = row+col neighbors on 32x32 grid)
    den[t] = sum_k ew[t,k] * ek[idx[t,k]]
    out = sigmoid(q) * num / den

Sharding: 8 cores = 4 batches x 2 head-pairs (128 features each). No collectives.

Per-core device algorithm (all matmul operands bf16, fp32 PSUM accumulation):
  - inputs stream on TWO parallel hardware DGE queues (sync -> Q1,
    scalar -> Q10), each in need-order:
      Q10: xt token-half 0 (256KB), wcol_e (256KB)
      Q1:  [wq|wkv] (192KB), xt token-half 1, wrow_e
    All transfers have 2KB DRAM/SBUF rows (23.5 vs 15.7 GB/s/engine for
    1KB rows); the host-expanded block-diagonal neighbor weights stream
    last and hide behind the PE phase.
    xt cols = h*1024 + kh*512 + tlo  (token t = h*512+tlo, d = kh*128+p).
  - the grid-transposed xt2 (cols kh*1024 + t', t' = c*32+r) is built
    ON-DEVICE by 4 strided copies (kh=1 on ACT, kh=0 on DVE), each gated
    on one xt half -- no HBM traffic for it.
  - q projected feature-major -> qT PSUM; ACT computes eq = exp(-qT).
  - k|v projected token-major per 128-token group (lhsT = xt slice) ->
    ek, kv; same from xt2 -> ekp, kvp (grid-col-major blocks).
  - The 63-neighbor gather+reduce decomposes into block-diagonal matmuls
    accumulated into zeroed PSUM (start=False + skip_group_check):
      row part: numT[f, tok-slice] += kv_g.T @ wrow_g   (16 matmuls)
      col part: numT[f, strided]   += kvp_g.T @ wcol_g  (16 matmuls,
                den first so the combine's recip chain overlaps them)
  - combine (DVE): m1 = (eq+1)*den [scalar_tensor_tensor] and
    rden = recip_approx(m1) for all quarters run DURING the col-num
    matmuls; only the final muls out = num*rden trail the PE
    (== sigmoid(q)*num/den).  Written feature-major bf16; host
    casts/transposes during unshard.  Output DMAs alternate between the
    sync and scalar DGE queues so descriptor issue is not serialized.
  - dummy matmuls warm the PE HAM clock gate while inputs stream in; a
    dummy Exp preloads the ACT table so no table load sits mid-kernel.
"""

import os
from contextlib import ExitStack

import ml_dtypes
import numpy as np

import concourse.bass as bass
import concourse.tile as tile
from concourse import bacc, mybir
from concourse.bass_utils import run_bass_kernel_spmd

BF = mybir.dt.bfloat16
F32 = mybir.dt.float32
AF = mybir.ActivationFunctionType
ALU = mybir.AluOpType

N = 32          # grid side
T = N * N       # tokens
D = 256         # model dim
F = 128         # features per core (2 heads x 64)
NEG = -1e30     # exp(NEG) == 0

LAST_RESULT = None  # BassKernelResults of the most recent run (for profiling)
_CACHED_NC = None


def _build_nc():
    nc = bacc.Bacc("TRN2", target_bir_lowering=False, debug=False)

    xt_ds = [nc.declare_dram_parameter(f"xth{i}", [128, 1024], BF,
                                       isOutput=False) for i in range(2)]
    wb1_d = nc.declare_dram_parameter("wb1", [128, 768], BF, isOutput=False)
    wrow_d = nc.declare_dram_parameter("wrow", [128, 1024], BF, isOutput=False)
    wcol_d = nc.declare_dram_parameter("wcol", [128, 1024], BF, isOutput=False)
    out_d = nc.declare_dram_parameter("out", [128, 1024], BF, isOutput=True)

    from concourse.tile_rust import add_dep_helper

    with tile.TileContext(nc) as tc, ExitStack() as ctx:
        sb = ctx.enter_context(tc.tile_pool(name="sb", bufs=1))
        ps_q = ctx.enter_context(tc.tile_pool(name="ps_q", bufs=1, space="PSUM"))
        ps_kv = ctx.enter_context(tc.tile_pool(name="ps_kv", bufs=3, space="PSUM"))
        ps_g = ctx.enter_context(tc.tile_pool(name="ps_g", bufs=1, space="PSUM"))

        xt = sb.tile([128, 2048], BF, tag="xt")
        xt2 = sb.tile([128, 2048], BF, tag="xt2")
        wb1 = sb.tile([128, 768], BF, tag="wb1")
        wrow_e = sb.tile([128, 1024], BF, tag="wrow_e")
        wcol_e = sb.tile([128, 1024], BF, tag="wcol_e")
        warm = sb.tile([128, 512], BF, tag="warm")
        pre = sb.tile([128, 16], F32, tag="pre")
        ek = sb.tile([128, 1024], BF, tag="ek")
        kv = sb.tile([128, 1024], BF, tag="kv")
        ekp = sb.tile([128, 1024], BF, tag="ekp")
        kvp = sb.tile([128, 1024], BF, tag="kvp")
        eq = sb.tile([128, 1024], F32, tag="eq")
        m1 = sb.tile([128, 1024], F32, tag="m1")
        rden = sb.tile([128, 1024], F32, tag="rden")
        w2 = sb.tile([128, 1024], BF, tag="w2")

        wq = wb1[:, 0:256]
        wkv = wb1[:, 256:768]

        # input loads split across BOTH hardware DGE queues (sync -> Q1,
        # scalar -> Q10) so the two streams run in parallel, each in
        # consumption-priority order; xt half h covers cols h*1024..
        nc.scalar.dma_start(out=xt[:, 0:1024], in_=xt_ds[0][:])
        nc.scalar.dma_start(out=wcol_e[:], in_=wcol_d[:])
        nc.sync.dma_start(out=wb1[:], in_=wb1_d[:])
        nc.sync.dma_start(out=xt[:, 1024:2048], in_=xt_ds[1][:])
        nc.sync.dma_start(out=wrow_e[:], in_=wrow_d[:])

        # grid-transpose copies for the kh=1 half of xt2 run on the ACT
        # engine's idle early window (their sources arrive first, on Q10);
        # the kh=0 half runs on DVE.  This keeps the DVE queue light so
        # the ekp/kvp muls (which gate the col-den matmuls and thus the
        # combine chain) run as early as possible.

        # PE warm-up: dummy matmuls while the input DMAs stream in, so the
        # HAM clock gate is released (1.2 -> 2.4 GHz) before the real work
        nc.gpsimd.memset(warm[:], 0.0)
        for i in range(9):
            # 7 medium then 2 small dummy matmuls: keep the PE continuously
            # busy from engine start until the first xt quarter lands, so
            # the clock-ramp activity window never resets
            wps = ps_kv.tile([128, 512], F32, tag="kvps")
            w_ = 256 if i < 7 else 128
            nc.tensor.matmul(wps[:, 0:w_], warm[:, 0:128], warm[:, 0:w_],
                             start=True, stop=True)

        # ACT table preload: a dummy Exp so the (only) table load happens
        # while inputs stream in
        nc.scalar.activation(pre[:], warm[:, 0:16], AF.Exp)

        # zero the grid accumulators on the (idle-early) DVE; every grid
        # matmul then accumulates with start=False + skip_group_check.
        # (den first -- its memsets gate the earliest grid matmuls; the
        # num memsets are interleaved after the xt2 copies below)
        numT = ps_g.tile([128, 1024], F32, tag="numT")
        denT = ps_g.tile([128, 1024], F32, tag="denT")
        for bank in range(2):
            nc.vector.memset(denT[:, bank * 512:(bank + 1) * 512], 0.0)

        # on-device grid transpose xt -> xt2 (t' = c*32 + r): 4 strided DVE
        # copies, each gated on one xt quarter transfer
        xt_cr = xt[:].rearrange("p (h kh r c) -> p h kh c r", h=2, kh=2, c=N)
        xt2_cr = xt2[:].rearrange("p (kh c r) -> p kh c r", kh=2, c=N)

        ek_vw = ek[:].rearrange("p (g f) -> p g f", f=128)
        kv_vw = kv[:].rearrange("p (g f) -> p g f", f=128)
        ekp_vw = ekp[:].rearrange("p (g f) -> p g f", f=128)
        kvp_vw = kvp[:].rearrange("p (g f) -> p g f", f=128)

        qp = {}

        def q_proj(nh):
            qp[nh] = ps_q.tile([128, 512], F32, name=f"qp{nh}", tag="qp")
            for kh in range(2):
                nc.tensor.matmul(
                    qp[nh][:],
                    wq[:, kh * 128:(kh + 1) * 128],
                    xt[:, nh * 1024 + kh * 512: nh * 1024 + (kh + 1) * 512],
                    start=(kh == 0), stop=(kh == 1),
                )

        def kv_proj(pr, src, ek_t, kv_t, do_mul=True):
            kvps = ps_kv.tile([128, 512], F32, tag="kvps")
            mm = {}
            for g2 in range(2):
                g = 2 * pr + g2
                for kh in range(2):
                    if src is xt:
                        base = (g // 4) * 1024 + kh * 512 + (g % 4) * 128
                    else:
                        base = kh * 1024 + g * 128
                    lhsT = src[:, base: base + 128]
                    mm[g2, kh] = nc.tensor.matmul(
                        kvps[:, g2 * 256:(g2 + 1) * 256],
                        lhsT,
                        wkv[:, kh * 256:(kh + 1) * 256],
                        start=(g2 == 0 and kh == 0),
                        stop=(g2 == 1 and kh == 1),
                    )
            # keep PSUM zero-region state machine ordering legal: the
            # start=True matmul first, the stop=True matmul last
            add_dep_helper(mm[1, 0].ins, mm[0, 0].ins, reason="psum start first")
            add_dep_helper(mm[1, 1].ins, mm[0, 1].ins, reason="psum stop last")
            kvps_v = kvps[:].rearrange("p (g c) -> p g c", g=2)
            ps_ = slice(2 * pr, 2 * pr + 2)
            nc.scalar.activation(ek_t[:, ps_, :], kvps_v[:, :, 0:128], AF.Exp)
            nc.vector.tensor_mul(kv_t[:, ps_, :], ek_t[:, ps_, :],
                                 kvps_v[:, :, 128:256])

        # PE order: q0 -> kvA pr0-1 -> kvA pr2-3 -> q1 -> kvB -> row -> col.
        # ACT order: expA0, eq0, expA1-3, eq1, expB0-3.
        # DVE order: memsets, xt2 copies + muls A interleaved, muls B,
        #            stt+recip (all quarters), final muls.
        # xt2 copies for kh=1 go FIRST on the ACT queue (their sources are
        # Q10's first two transfers, landing before any exp input exists);
        # the kh=0 copies run on DVE
        nc.scalar.copy(xt2_cr[:, 1, :, 0:16], xt_cr[:, 0, 1, :, :])
        nc.scalar.copy(xt2_cr[:, 1, :, 16:32], xt_cr[:, 1, 1, :, :])
        q_proj(0)
        kv_proj(0, xt, ek_vw, kv_vw)
        nc.scalar.activation(eq[:, 0:512], qp[0][:], AF.Exp, scale=-1.0)
        kv_proj(1, xt, ek_vw, kv_vw)
        nc.vector.tensor_copy(xt2_cr[:, 0, :, 0:16], xt_cr[:, 0, 0, :, :])
        kv_proj(2, xt, ek_vw, kv_vw)
        nc.vector.tensor_copy(xt2_cr[:, 0, :, 16:32], xt_cr[:, 1, 0, :, :])
        for bank in range(2):
            nc.vector.memset(numT[:, bank * 512:(bank + 1) * 512], 0.0)
        kv_proj(3, xt, ek_vw, kv_vw)
        q_proj(1)
        for pr in range(4):
            kv_proj(pr, xt2, ekp_vw, kvp_vw)
        nc.scalar.activation(eq[:, 512:1024], qp[1][:], AF.Exp, scale=-1.0)

        # grid reduction, den parts FIRST (row den, col den, row num,
        # col num) so the combine's den->recip chain overlaps the num
        # matmuls.  Row part writes contiguous out cols per 4-grid-row
        # slice; col part writes strided out cols (token r*32+c).
        GK = dict(start=False, stop=False, skip_group_check=True)
        numT_v = numT[:].rearrange("p (r c) -> p c r", c=N)
        denT_v = denT[:].rearrange("p (r c) -> p c r", c=N)
        wcol_gv = wcol_e[:].rearrange("p (g cb r) -> p g cb r", g=8, cb=4)
        for g in range(8):
            gs = slice(g * 128, (g + 1) * 128)
            nc.tensor.matmul(denT[:, gs], ek[:, gs], wrow_e[:, gs], **GK)
        for g in range(8):
            gs = slice(g * 128, (g + 1) * 128)
            nc.tensor.matmul(denT_v[:, 4 * g:4 * (g + 1), :],
                             ekp[:, gs], wcol_gv[:, g], **GK)
        for g in range(8):
            gs = slice(g * 128, (g + 1) * 128)
            nc.tensor.matmul(numT[:, gs], kv[:, gs], wrow_e[:, gs], **GK)
        for g in range(8):
            gs = slice(g * 128, (g + 1) * 128)
            nc.tensor.matmul(numT_v[:, 4 * g:4 * (g + 1), :],
                             kvp[:, gs], wcol_gv[:, g], **GK)

        # combine: out = num * recip(den * (1 + exp(-q))) == sigmoid(q)*num/den
        # in halves: stt+recip (den-gated) overlap the num matmuls; only
        # the two final muls trail the PE, each feeding its output DMA.
        for hf in range(2):
            hs = slice(hf * 512, (hf + 1) * 512)
            nc.vector.scalar_tensor_tensor(
                m1[:, hs], eq[:, hs], 1.0, denT[:, hs], ALU.add, ALU.mult)
            nc.vector.reciprocal_approx_fast(rden[:, hs], m1[:, hs])
        for qt in range(4):
            hs = slice(qt * 256, (qt + 1) * 256)
            nc.vector.tensor_mul(w2[:, hs], rden[:, hs], numT[:, hs])
            eng = nc.sync if qt % 2 == 0 else nc.scalar
            eng.dma_start(out=out_d[:, hs], in_=w2[:, hs])

    nc.compile()
    return nc


def _get_nc():
    global _CACHED_NC
    if _CACHED_NC is None:
        _CACHED_NC = _build_nc()
    return _CACHED_NC


def _interleave_halves(a):
    """[256, M] -> [128, 2*M] with cols (half, m); partitions = dim%128."""
    return np.concatenate([a[0:128], a[128:256]], axis=1)


def make_shards(x, Wq, Wk, Wv, wbias):
    """Build the per-core input maps (host-side layout/sharding only)."""
    bf = ml_dtypes.bfloat16
    B = x.shape[0]

    # neighbor-weight reorganization: for token t=(r,c), sorted wbias cols are
    #   [0, r)   -> col-neighbor grid-row j = pos
    #   [r, r+N) -> row-neighbor grid-col j = pos - r
    #   [r+N, 2N-1) -> col-neighbor grid-row j = pos - (N - 1)
    Wr = np.empty((T, N), np.float32)
    Wc = np.full((T, N), NEG, np.float32)
    for t in range(T):
        r = t // N
        Wr[t] = wbias[t, r:r + N]
        Wc[t, :r] = wbias[t, :r]
        Wc[t, r + 1:] = wbias[t, r + N:]

    # block-diagonal expanded weights (exp-folded; off-block entries 0):
    #   wrow[rb*32+j, g*128 + rb*32 + c] = exp(Wr[(4g+rb)*32 + c, j])
    #   wcol[cb*32+j, g*128 + cb*32 + r] = exp(Wc[r*32 + 4g+cb, j])
    wrow = np.full((128, 1024), NEG, np.float32)
    wcol = np.full((128, 1024), NEG, np.float32)
    rb, j, c = np.meshgrid(np.arange(4), np.arange(N), np.arange(N),
                           indexing="ij")
    for g in range(8):
        wrow[rb * N + j, g * 128 + rb * N + c] = Wr[(4 * g + rb) * N + c, j]
        wcol[rb * N + j, g * 128 + rb * N + c] = Wc[c * N + 4 * g + rb, j]
    wrow = np.exp(wrow).astype(bf)
    wcol = np.exp(wcol).astype(bf)

    xt_b = []
    for b in range(B):
        xtb = np.ascontiguousarray(_interleave_halves(x[b].T)).astype(bf)
        xt_b.append(xtb)

    in_maps = []
    for core in range(8):
        b, hp = core // 2, core % 2
        sl = slice(hp * 128, (hp + 1) * 128)
        wq_c = _interleave_halves(Wq[sl].T).astype(bf)                # [128,256]
        k_h = Wk[sl].T.reshape(2, 128, 128)
        v_h = Wv[sl].T.reshape(2, 128, 128)
        wkv_c = np.concatenate([k_h[0], v_h[0], k_h[1], v_h[1]],
                               axis=1).astype(bf)                      # [128,512]
        wb1_c = np.ascontiguousarray(
            np.concatenate([wq_c, wkv_c], axis=1))                     # [128,768]
        xh = xt_b[b].reshape(128, 2, 2, 512).transpose(0, 2, 1, 3)
        xh = xh.reshape(128, 2048)
        in_maps.append({
            "xth0": np.ascontiguousarray(xh[:, 0:1024]),
            "xth1": np.ascontiguousarray(xh[:, 1024:2048]),
            "wb1": wb1_c,
            "wrow": wrow,
            "wcol": wcol,
        })
    return in_maps


def kernel(x, Wq, Wk, Wv, wbias, key_indices=None, **_unused):
    global LAST_RESULT
    x = np.asarray(x, np.float32)
    Wq = np.asarray(Wq, np.float32)
    Wk = np.asarray(Wk, np.float32)
    Wv = np.asarray(Wv, np.float32)
    wbias = np.asarray(wbias, np.float32)

    nc = _get_nc()
    in_maps = make_shards(x, Wq, Wk, Wv, wbias)
    try:
        res = run_bass_kernel_spmd(nc, in_maps, core_ids=list(range(8)))
    except ModuleNotFoundError:
        # BASS_TRACE set but the NTFF profile hook module is unavailable in
        # this environment -- rerun untraced
        os.environ["BASS_NEVER_TRACE"] = "1"
        res = run_bass_kernel_spmd(nc, in_maps, core_ids=list(range(8)))
    LAST_RESULT = res

    B = x.shape[0]
    out = np.empty((B, T, D), np.float32)
    for core in range(8):
        b, hp = core // 2, core % 2
        out[b, :, hp * 128:(hp + 1) * 128] = \
            res.results[core]["out"].astype(np.float32).T
    return out
